# revision 2
# baseline (speedup 1.0000x reference)
"""LSTM autoencoder Bass kernel v2 for Trainium2, 8 NeuronCores.

Latency-optimized vs v1: the wall-clock is bound by the serial per-step
dependency chain (MM -> sigmoid -> c-update -> tanh -> h-mul -> MM), so
every tile keeps all 128 partitions busy with the minimum free-dim:

  Encoder (per stream of 256 batch): 8 chunks x 16 units = 128 partition
  rows; gates in column blocks [f|i|o|g] of 32 cols -> PSUM G [128,128].
  Per gate: one x-matmul (with a ones-row carrying the bias) and one
  block-diagonal h-matmul. One sigmoid over all gates (g-gate preact
  prescaled x2 so tanh(g) = 2*sig(2g)-1), then a 3-op fused DVE c-update:
    U2 = (Sg - 0.5)*Si ; C2 = Sf*c ; c = 2*U2 + C2
  one small Tanh(c), one h = T2*So.

  Decoder (input constant over time): 16 chunks x 8 units = 128 rows,
  gates in 16-col blocks -> G [128,64]. xgd = Wxd*h_enc + b precomputed
  once per stream by 8 accumulating matmuls straight out of the encoder
  h layout (no partition remap). Per step: identity-matmul PSUM-init
  with xgd, 4 h-matmuls, sigmoid, 3-op c-update, Tanh, h-mul, and a
  y-matmul accumulating 4 timesteps into one PSUM tile drained by a DVE
  bias-add into a big SBUF buffer.

  DMA: x is preloaded per stream as [65, T*32] bf16 (4 chunked DMAs);
  y leaves as [128, T/4*64] bf16 in 4 chunked DMAs per stream. No
  per-step DMAs anywhere.
"""
import sys
if "/opt/trn_rl_repo" not in sys.path:
    sys.path.insert(0, "/opt/trn_rl_repo")

import numpy as np
import ml_dtypes

BF = ml_dtypes.bfloat16

SEQ_LEN = 256
NF = 8
HID = 16
BATCH = 4096
N_CORES = 8
CB = BATCH // N_CORES      # 512
NSTREAM = 2
SB = CB // NSTREAM         # 256
ENC_NC = 8                 # enc chunks/stream
ENC_F = SB // ENC_NC       # 32
DEC_NC = 16
DEC_F = SB // DEC_NC       # 16

# gate column-block order; pytorch row offsets (i,f,g,o)
GORD = ["f", "i", "o", "g"]
OFF_E = {"i": 0, "f": HID, "g": 2 * HID, "o": 3 * HID}
OFF_D = {"i": 0, "f": NF, "g": 2 * NF, "o": 3 * NF}

XROWS = ENC_NC * NF + 1    # 65 (ones row at 64)

# weight blob column offsets (bf16 blob [128, WCOLS])
#   whe: 4 x 128 ; wxe: 4 x 128 (rows 0:65) ; whd: 4 x 128 ;
#   wxgd: 8 x 128 ; wy: 128 ; ident: 128
O_WHE = 0
O_WXE = O_WHE + 4 * 128
O_WHD = O_WXE + 4 * 128
O_WXGD = O_WHD + 4 * 128
O_WY = O_WXGD + 8 * 128
O_ID = O_WY + 128
O_Z = O_ID + 128
WCOLS = O_Z + 128


def pack_weights(enc_Wih, enc_Whh, enc_bih, enc_bhh,
                 dec_Wih, dec_Whh, dec_bih, dec_bhh, out_W, out_b):
    wb = np.zeros((128, WCOLS), dtype=np.float32)
    be = enc_bih + enc_bhh
    for gi, gn in enumerate(GORD):
        s = 2.0 if gn == "g" else 1.0
        for q in range(ENC_NC):
            for u in range(HID):
                m = q * HID + u
                row = OFF_E[gn] + u
                wb[q * HID:(q + 1) * HID, O_WHE + gi * 128 + m] = \
                    s * enc_Whh[row, :]
                wb[q * NF:(q + 1) * NF, O_WXE + gi * 128 + m] = \
                    s * enc_Wih[row, :]
                wb[ENC_NC * NF, O_WXE + gi * 128 + m] = s * be[row]
    for gi, gn in enumerate(GORD):
        s = 2.0 if gn == "g" else 1.0
        for q in range(DEC_NC):
            for u in range(NF):
                m = q * NF + u
                row = OFF_D[gn] + u
                wb[q * NF:(q + 1) * NF, O_WHD + gi * 128 + m] = \
                    s * dec_Whh[row, :]
    # xgd: out rows (qh, du), 8 matmuls indexed (gi, jh); rhs = H[:,16jh:+16]
    # lhsT[(q,eu), (qh,du)] = s*dec_Wih[off+du, eu] if qh == 2q+jh
    for gi, gn in enumerate(GORD):
        s = 2.0 if gn == "g" else 1.0
        for jh in range(2):
            col0 = O_WXGD + (gi * 2 + jh) * 128
            for q in range(ENC_NC):
                qh = 2 * q + jh
                for du in range(NF):
                    m = qh * NF + du
                    wb[q * HID:(q + 1) * HID, col0 + m] = \
                        s * dec_Wih[OFF_D[gn] + du, :]
    # y: lhsT[(q,du), (q,f)] = out_W[f, du]
    for q in range(DEC_NC):
        for u in range(NF):
            k = q * NF + u
            for f in range(NF):
                wb[k, O_WY + q * NF + f] = out_W[f, u]
    wb[:, O_ID:O_ID + 128] = np.eye(128, dtype=np.float32)

    # f32 blob [128, 65]: b_dec [128,64] then by [128,1]
    wf = np.zeros((128, 65), dtype=np.float32)
    bd = dec_bih + dec_bhh
    for gi, gn in enumerate(GORD):
        s = 2.0 if gn == "g" else 1.0
        for q in range(DEC_NC):
            for du in range(NF):
                wf[q * NF + du, gi * DEC_F:(gi + 1) * DEC_F] = \
                    s * bd[OFF_D[gn] + du]
    for q in range(DEC_NC):
        for f in range(NF):
            wf[q * NF + f, 64] = out_b[f]
    return wb.astype(BF), wf


def prep_x(x, T):
    """x [BATCH,T,NF] f32 -> per-core [NSTREAM, 65, T*ENC_F] bf16."""
    out = []
    for c in range(N_CORES):
        xc = x[c * CB:(c + 1) * CB]
        X = np.empty((NSTREAM, XROWS, T * ENC_F), dtype=np.float32)
        for s in range(NSTREAM):
            xs = xc[s * SB:(s + 1) * SB]          # [256, T, 8]
            v = xs.reshape(ENC_NC, ENC_F, T, NF)  # q, j, t, f
            v = v.transpose(0, 3, 2, 1)           # q, f, t, j
            X[s, :ENC_NC * NF] = v.reshape(ENC_NC * NF, T * ENC_F)
            X[s, ENC_NC * NF] = 1.0
        out.append(X.astype(BF))
    return out


def assemble_y(ydevs, T):
    """per-core ydev [NSTREAM, 128, (T//4)*64] bf16 -> y [BATCH,T,NF] f32."""
    y = np.empty((BATCH, T, NF), dtype=np.float32)
    for c, yd in enumerate(ydevs):
        v = yd.astype(np.float32).reshape(NSTREAM, DEC_NC, NF, T // 4, 4, DEC_F)
        # rows (qh, f), cols (tg, j, jj): batch = s*SB + qh*16 + jj
        v = v.transpose(0, 1, 5, 3, 4, 2)   # s, qh, jj, tg, j, f
        y[c * CB:(c + 1) * CB] = v.reshape(CB, T, NF)
    return y


def build_program(T=SEQ_LEN):
    import concourse.bass as bass
    import concourse.bacc as bacc
    import concourse.tile as tile
    from concourse import mybir
    from contextlib import ExitStack

    F32 = mybir.dt.float32
    BF16 = mybir.dt.bfloat16
    SIG = mybir.ActivationFunctionType.Sigmoid
    TANH = mybir.ActivationFunctionType.Tanh
    MULT = mybir.AluOpType.mult
    ADD = mybir.AluOpType.add
    SUB = mybir.AluOpType.subtract

    nc = bacc.Bacc("TRN2", target_bir_lowering=False, debug=False)

    NG = T // 4
    xdev = nc.dram_tensor("xdev", [NSTREAM, XROWS, T * ENC_F], BF16,
                          kind="ExternalInput")
    wblob = nc.dram_tensor("wblob", [128, WCOLS], BF16, kind="ExternalInput")
    wf32 = nc.dram_tensor("wf32", [128, 65], F32, kind="ExternalInput")
    ydev = nc.dram_tensor("ydev", [NSTREAM, 128, NG * 64], BF16,
                          kind="ExternalOutput")

    with tile.TileContext(nc) as tc, ExitStack() as ctx:
        wp = ctx.enter_context(tc.tile_pool(name="weights", bufs=1))
        xp = ctx.enter_context(tc.tile_pool(name="xbuf", bufs=1))
        st = ctx.enter_context(tc.tile_pool(name="state", bufs=1))
        yb = ctx.enter_context(tc.tile_pool(name="ybuf", bufs=1))
        sp = ctx.enter_context(tc.tile_pool(name="scratch", bufs=2))
        gp = ctx.enter_context(tc.tile_pool(name="gpsum", bufs=1, space="PSUM"))
        yp = ctx.enter_context(tc.tile_pool(name="ypsum", bufs=2, space="PSUM"))

        WB = wp.tile([128, WCOLS], BF16, tag="wb")
        WF = wp.tile([128, 65], F32, tag="wf")
        nc.sync.dma_start(WB[:], wblob[:])
        nc.sync.dma_start(WF[:], wf32[:])

        X = [xp.tile([XROWS, T * ENC_F], BF16, tag=f"X{s}", name=f"X{s}")
             for s in range(NSTREAM)]
        for s in range(NSTREAM):
            ncols = T * ENC_F
            for h in range(4):
                c0, c1 = h * ncols // 4, (h + 1) * ncols // 4
                nc.sync.dma_start(X[s][:, c0:c1], xdev[s, :, c0:c1])

        Ybuf = [yb.tile([128, NG * 64], BF16, tag=f"Yb{s}", name=f"Yb{s}")
                for s in range(NSTREAM)]

        H = [st.tile([128, ENC_F], BF16, tag=f"H{s}", name=f"H{s}")
             for s in range(NSTREAM)]
        C = [st.tile([128, ENC_F], BF16, tag=f"C{s}", name=f"C{s}")
             for s in range(NSTREAM)]
        for s in range(NSTREAM):
            nc.vector.memset(H[s][:], 0.0)
            nc.vector.memset(C[s][:], 0.0)

        def lT(base, i):
            return WB[:, base + i * 128: base + (i + 1) * 128]

        # ---------------- encoder ----------------
        for t in range(T):
            for s in range(NSTREAM):
                G = gp.tile([128, 128], F32, tag=f"G{s}", name=f"G{s}")
                xsl = X[s][0:XROWS, t * ENC_F:(t + 1) * ENC_F]
                # one accumulation group per step: first x-matmul opens it
                # (region goes pending-zero; first touch of each address
                # overwrites), last h-matmul closes it
                for gi in range(4):
                    nc.tensor.matmul(G[:, gi * ENC_F:(gi + 1) * ENC_F],
                                     lT(O_WXE, gi)[0:XROWS, :], xsl,
                                     start=(gi == 0), stop=False,
                                     tile_position=(0, 0))
                for gi in range(4):
                    nc.tensor.matmul(G[:, gi * ENC_F:(gi + 1) * ENC_F],
                                     lT(O_WHE, gi), H[s][:],
                                     start=False, stop=(gi == 3),
                                     tile_position=(0, 0))
                S = sp.tile([128, 128], BF16, tag=f"S{s}")
                nc.scalar.activation(S[:], G[:], SIG)
                U2 = sp.tile([128, ENC_F], BF16, tag=f"U2{s}")
                nc.vector.scalar_tensor_tensor(
                    U2[:], S[:, 96:128], 0.5, S[:, 32:64], SUB, MULT)
                C2 = sp.tile([128, ENC_F], BF16, tag=f"C2{s}")
                nc.vector.tensor_mul(C2[:], S[:, 0:32], C[s][:])
                nc.vector.scalar_tensor_tensor(
                    C[s][:], U2[:], 2.0, C2[:], MULT, ADD)
                T2 = sp.tile([128, ENC_F], BF16, tag=f"T2{s}")
                nc.scalar.activation(T2[:], C[s][:], TANH)
                nc.vector.tensor_mul(H[s][:], T2[:], S[:, 64:96])

        # ---------------- enc->dec: xgd ----------------
        XG = [st.tile([128, 64], BF16, tag=f"XG{s}", name=f"XG{s}")
              for s in range(NSTREAM)]
        Hd = [st.tile([128, DEC_F], BF16, tag=f"Hd{s}", name=f"Hd{s}")
              for s in range(NSTREAM)]
        Cd = [st.tile([128, DEC_F], BF16, tag=f"Cd{s}", name=f"Cd{s}")
              for s in range(NSTREAM)]
        for s in range(NSTREAM):
            XGP = gp.tile([128, 128], F32, tag=f"G{s}", name=f"XGP{s}")
            for gi in range(4):
                for jh in range(2):
                    nc.tensor.matmul(
                        XGP[:, gi * DEC_F:(gi + 1) * DEC_F],
                        lT(O_WXGD, gi * 2 + jh),
                        H[s][:, jh * DEC_F:(jh + 1) * DEC_F],
                        start=(jh == 0), stop=(jh == 1),
                        tile_position=(0, 0))
            nc.vector.tensor_add(XG[s][:], XGP[:, 0:64], WF[:, 0:64])
            nc.vector.memset(Hd[s][:], 0.0)
            nc.vector.memset(Cd[s][:], 0.0)

        # ---------------- decoder ----------------
        Y = [None] * NSTREAM
        for t in range(T):
            j = t % 4
            tg = t // 4
            for s in range(NSTREAM):
                G = gp.tile([128, 64], F32, tag=f"Gd{s}", name=f"Gd{s}")
                nc.tensor.matmul(G[:], lT(O_ID, 0), XG[s][:],
                                 start=True, stop=False, tile_position=(0, 0))
                for gi in range(4):
                    nc.tensor.matmul(G[:, gi * DEC_F:(gi + 1) * DEC_F],
                                     lT(O_WHD, gi), Hd[s][:],
                                     start=False, stop=(gi == 3),
                                     tile_position=(0, 0))
                S = sp.tile([128, 64], BF16, tag=f"Sd{s}")
                nc.scalar.activation(S[:], G[:], SIG)
                U2 = sp.tile([128, DEC_F], BF16, tag=f"U2d{s}")
                nc.vector.scalar_tensor_tensor(
                    U2[:], S[:, 48:64], 0.5, S[:, 16:32], SUB, MULT)
                C2 = sp.tile([128, DEC_F], BF16, tag=f"C2d{s}")
                nc.vector.tensor_mul(C2[:], S[:, 0:16], Cd[s][:])
                nc.vector.scalar_tensor_tensor(
                    Cd[s][:], U2[:], 2.0, C2[:], MULT, ADD)
                T2 = sp.tile([128, DEC_F], BF16, tag=f"T2d{s}")
                nc.scalar.activation(T2[:], Cd[s][:], TANH)
                nc.vector.tensor_mul(Hd[s][:], T2[:], S[:, 32:48])
                if j == 0:
                    Y[s] = yp.tile([128, 64], F32, tag=f"Y{s}", name=f"Y{s}")
                nc.tensor.matmul(Y[s][:, j * DEC_F:(j + 1) * DEC_F],
                                 lT(O_WY, 0), Hd[s][:],
                                 start=True, stop=True, tile_position=(0, 0))
                if j == 3:
                    nc.vector.tensor_scalar_add(
                        Ybuf[s][:, tg * 64:(tg + 1) * 64], Y[s][:],
                        WF[:, 64:65])
                    if (tg + 1) % (NG // 4) == 0:
                        h = (tg + 1) // (NG // 4) - 1
                        c0 = h * (NG // 4) * 64
                        c1 = (h + 1) * (NG // 4) * 64
                        nc.sync.dma_start(ydev[s, :, c0:c1],
                                          Ybuf[s][:, c0:c1])

    nc.compile()
    return nc


_cached = {}
TRACE = False
RUN_KWARGS = {}
LAST_RESULT = None


def _get_program(T=SEQ_LEN):
    if T not in _cached:
        _cached[T] = build_program(T)
    return _cached[T]


def kernel(x, enc_Wih, enc_Whh, enc_bih, enc_bhh,
           dec_Wih, dec_Whh, dec_bih, dec_bhh, out_W, out_b):
    from concourse.bass_utils import run_bass_kernel_spmd

    x = np.asarray(x, dtype=np.float32)
    T = x.shape[1]
    nc = _get_program(T)

    wb, wf = pack_weights(
        np.asarray(enc_Wih), np.asarray(enc_Whh),
        np.asarray(enc_bih), np.asarray(enc_bhh),
        np.asarray(dec_Wih), np.asarray(dec_Whh),
        np.asarray(dec_bih), np.asarray(dec_bhh),
        np.asarray(out_W), np.asarray(out_b))
    xdevs = prep_x(x, T)
    in_maps = [{"xdev": xdevs[c], "wblob": wb, "wf32": wf}
               for c in range(N_CORES)]
    res = run_bass_kernel_spmd(nc, in_maps, core_ids=list(range(N_CORES)),
                               trace=TRACE, **RUN_KWARGS)
    global LAST_RESULT
    LAST_RESULT = res
    return assemble_y([r["ydev"] for r in res.results], T)


# revision 3
# speedup vs baseline: 1.0551x; 1.0551x over previous
"""LSTM autoencoder Bass kernel v2 for Trainium2, 8 NeuronCores.

Latency-optimized vs v1: the wall-clock is bound by the serial per-step
dependency chain (MM -> sigmoid -> c-update -> tanh -> h-mul -> MM), so
every tile keeps all 128 partitions busy with the minimum free-dim:

  Encoder (per stream of 256 batch): 8 chunks x 16 units = 128 partition
  rows; gates in column blocks [f|i|o|g] of 32 cols -> PSUM G [128,128].
  Per gate: one x-matmul (with a ones-row carrying the bias) and one
  block-diagonal h-matmul. One sigmoid over all gates (g-gate preact
  prescaled x2 so tanh(g) = 2*sig(2g)-1), then a 3-op fused DVE c-update:
    U2 = (Sg - 0.5)*Si ; C2 = Sf*c ; c = 2*U2 + C2
  one small Tanh(c), one h = T2*So.

  Decoder (input constant over time): 16 chunks x 8 units = 128 rows,
  gates in 16-col blocks -> G [128,64]. xgd = Wxd*h_enc + b precomputed
  once per stream by 8 accumulating matmuls straight out of the encoder
  h layout (no partition remap). Per step: identity-matmul PSUM-init
  with xgd, 4 h-matmuls, sigmoid, 3-op c-update, Tanh, h-mul, and a
  y-matmul accumulating 4 timesteps into one PSUM tile drained by a DVE
  bias-add into a big SBUF buffer.

  DMA: x is preloaded per stream as [65, T*32] bf16 (4 chunked DMAs);
  y leaves as [128, T/4*64] bf16 in 4 chunked DMAs per stream. No
  per-step DMAs anywhere.
"""
import sys
if "/opt/trn_rl_repo" not in sys.path:
    sys.path.insert(0, "/opt/trn_rl_repo")

import numpy as np
import ml_dtypes

BF = ml_dtypes.bfloat16

SEQ_LEN = 256
NF = 8
HID = 16
BATCH = 4096
N_CORES = 8
CB = BATCH // N_CORES      # 512
NSTREAM = 2
SB = CB // NSTREAM         # 256
ENC_NC = 8                 # enc chunks/stream
ENC_F = SB // ENC_NC       # 32
DEC_NC = 16
DEC_F = SB // DEC_NC       # 16

# gate column-block order; pytorch row offsets (i,f,g,o)
GORD = ["f", "i", "g", "o"]
OFF_E = {"i": 0, "f": HID, "g": 2 * HID, "o": 3 * HID}
OFF_D = {"i": 0, "f": NF, "g": 2 * NF, "o": 3 * NF}

XROWS = ENC_NC * NF + 1    # 65 (ones row at 64)

# weight blob column offsets (bf16 blob [128, WCOLS])
#   whe: 4 x 128 ; wxe: 4 x 128 (rows 0:65) ; whd: 4 x 128 ;
#   wxgd: 8 x 128 ; wy: 128 ; ident: 128
O_WHE = 0
O_WXE = O_WHE + 4 * 128
O_WHD = O_WXE + 4 * 128
O_WXGD = O_WHD + 4 * 128
O_WY = O_WXGD + 8 * 128
O_ID = O_WY + 128
O_Z = O_ID + 128
WCOLS = O_Z + 128


def pack_weights(enc_Wih, enc_Whh, enc_bih, enc_bhh,
                 dec_Wih, dec_Whh, dec_bih, dec_bhh, out_W, out_b):
    wb = np.zeros((128, WCOLS), dtype=np.float32)
    be = enc_bih + enc_bhh
    for gi, gn in enumerate(GORD):
        s = 2.0 if gn == "g" else 1.0
        for q in range(ENC_NC):
            for u in range(HID):
                m = q * HID + u
                row = OFF_E[gn] + u
                wb[q * HID:(q + 1) * HID, O_WHE + gi * 128 + m] = \
                    s * enc_Whh[row, :]
                wb[q * NF:(q + 1) * NF, O_WXE + gi * 128 + m] = \
                    s * enc_Wih[row, :]
                wb[ENC_NC * NF, O_WXE + gi * 128 + m] = s * be[row]
    for gi, gn in enumerate(GORD):
        s = 2.0 if gn == "g" else 1.0
        for q in range(DEC_NC):
            for u in range(NF):
                m = q * NF + u
                row = OFF_D[gn] + u
                wb[q * NF:(q + 1) * NF, O_WHD + gi * 128 + m] = \
                    s * dec_Whh[row, :]
    # xgd: out rows (qh, du), 8 matmuls indexed (gi, jh); rhs = H[:,16jh:+16]
    # lhsT[(q,eu), (qh,du)] = s*dec_Wih[off+du, eu] if qh == 2q+jh
    for gi, gn in enumerate(GORD):
        s = 2.0 if gn == "g" else 1.0
        for jh in range(2):
            col0 = O_WXGD + (gi * 2 + jh) * 128
            for q in range(ENC_NC):
                qh = 2 * q + jh
                for du in range(NF):
                    m = qh * NF + du
                    wb[q * HID:(q + 1) * HID, col0 + m] = \
                        s * dec_Wih[OFF_D[gn] + du, :]
    # y: lhsT[(q,du), (q,f)] = out_W[f, du]
    for q in range(DEC_NC):
        for u in range(NF):
            k = q * NF + u
            for f in range(NF):
                wb[k, O_WY + q * NF + f] = out_W[f, u]
    wb[:, O_ID:O_ID + 128] = np.eye(128, dtype=np.float32)

    # f32 blob [128, 65]: b_dec [128,64] then by [128,1]
    wf = np.zeros((128, 65), dtype=np.float32)
    bd = dec_bih + dec_bhh
    for gi, gn in enumerate(GORD):
        s = 2.0 if gn == "g" else 1.0
        for q in range(DEC_NC):
            for du in range(NF):
                wf[q * NF + du, gi * DEC_F:(gi + 1) * DEC_F] = \
                    s * bd[OFF_D[gn] + du]
    for q in range(DEC_NC):
        for f in range(NF):
            wf[q * NF + f, 64] = out_b[f]
    return wb.astype(BF), wf


def prep_x(x, T):
    """x [BATCH,T,NF] f32 -> per-core [NSTREAM, 65, T*ENC_F] bf16."""
    out = []
    for c in range(N_CORES):
        xc = x[c * CB:(c + 1) * CB]
        X = np.empty((NSTREAM, XROWS, T * ENC_F), dtype=np.float32)
        for s in range(NSTREAM):
            xs = xc[s * SB:(s + 1) * SB]          # [256, T, 8]
            v = xs.reshape(ENC_NC, ENC_F, T, NF)  # q, j, t, f
            v = v.transpose(0, 3, 2, 1)           # q, f, t, j
            X[s, :ENC_NC * NF] = v.reshape(ENC_NC * NF, T * ENC_F)
            X[s, ENC_NC * NF] = 1.0
        out.append(X.astype(BF))
    return out


def assemble_y(ydevs, T):
    """per-core ydev [NSTREAM, 128, (T//4)*64] bf16 -> y [BATCH,T,NF] f32."""
    y = np.empty((BATCH, T, NF), dtype=np.float32)
    for c, yd in enumerate(ydevs):
        v = yd.astype(np.float32).reshape(NSTREAM, DEC_NC, NF, T // 4, 4, DEC_F)
        # rows (qh, f), cols (tg, j, jj): batch = s*SB + qh*16 + jj
        v = v.transpose(0, 1, 5, 3, 4, 2)   # s, qh, jj, tg, j, f
        y[c * CB:(c + 1) * CB] = v.reshape(CB, T, NF)
    return y


def build_program(T=SEQ_LEN):
    import concourse.bass as bass
    import concourse.bacc as bacc
    import concourse.tile as tile
    from concourse import mybir
    from contextlib import ExitStack

    F32 = mybir.dt.float32
    BF16 = mybir.dt.bfloat16
    SIG = mybir.ActivationFunctionType.Sigmoid
    TANH = mybir.ActivationFunctionType.Tanh
    MULT = mybir.AluOpType.mult
    ADD = mybir.AluOpType.add
    SUB = mybir.AluOpType.subtract

    nc = bacc.Bacc("TRN2", target_bir_lowering=False, debug=False)

    NG = T // 4
    xdev = nc.dram_tensor("xdev", [NSTREAM, XROWS, T * ENC_F], BF16,
                          kind="ExternalInput")
    wblob = nc.dram_tensor("wblob", [128, WCOLS], BF16, kind="ExternalInput")
    wf32 = nc.dram_tensor("wf32", [128, 65], F32, kind="ExternalInput")
    ydev = nc.dram_tensor("ydev", [NSTREAM, 128, NG * 64], BF16,
                          kind="ExternalOutput")

    with tile.TileContext(nc) as tc, ExitStack() as ctx:
        wp = ctx.enter_context(tc.tile_pool(name="weights", bufs=1))
        xp = ctx.enter_context(tc.tile_pool(name="xbuf", bufs=1))
        st = ctx.enter_context(tc.tile_pool(name="state", bufs=1))
        yb = ctx.enter_context(tc.tile_pool(name="ybuf", bufs=1))
        sp = ctx.enter_context(tc.tile_pool(name="scratch", bufs=2))
        gp = ctx.enter_context(tc.tile_pool(name="gpsum", bufs=1, space="PSUM"))
        yp = ctx.enter_context(tc.tile_pool(name="ypsum", bufs=2, space="PSUM"))

        WB = wp.tile([128, WCOLS], BF16, tag="wb")
        WF = wp.tile([128, 65], F32, tag="wf")
        nc.sync.dma_start(WB[:], wblob[:])
        nc.sync.dma_start(WF[:], wf32[:])

        X = [xp.tile([XROWS, T * ENC_F], BF16, tag=f"X{s}", name=f"X{s}")
             for s in range(NSTREAM)]
        for s in range(NSTREAM):
            ncols = T * ENC_F
            for h in range(4):
                c0, c1 = h * ncols // 4, (h + 1) * ncols // 4
                nc.sync.dma_start(X[s][:, c0:c1], xdev[s, :, c0:c1])

        Ybuf = [yb.tile([128, NG * 64], BF16, tag=f"Yb{s}", name=f"Yb{s}")
                for s in range(NSTREAM)]

        H = [st.tile([128, ENC_F], BF16, tag=f"H{s}", name=f"H{s}")
             for s in range(NSTREAM)]
        C = [st.tile([128, ENC_F], BF16, tag=f"C{s}", name=f"C{s}")
             for s in range(NSTREAM)]
        for s in range(NSTREAM):
            nc.vector.memset(H[s][:], 0.0)
            nc.vector.memset(C[s][:], 0.0)

        def lT(base, i):
            return WB[:, base + i * 128: base + (i + 1) * 128]

        # ---------------- encoder ----------------
        for t in range(T):
            for s in range(NSTREAM):
                G = gp.tile([128, 128], F32, tag=f"G{s}", name=f"G{s}")
                xsl = X[s][0:XROWS, t * ENC_F:(t + 1) * ENC_F]
                # one accumulation group per step: first x-matmul opens it
                # (region goes pending-zero; first touch of each address
                # overwrites), last h-matmul closes it
                for gi in range(4):
                    nc.tensor.matmul(G[:, gi * ENC_F:(gi + 1) * ENC_F],
                                     lT(O_WXE, gi)[0:XROWS, :], xsl,
                                     start=(gi == 0), stop=False,
                                     tile_position=(0, 0))
                for gi in range(4):
                    nc.tensor.matmul(G[:, gi * ENC_F:(gi + 1) * ENC_F],
                                     lT(O_WHE, gi), H[s][:],
                                     start=False, stop=(gi == 3),
                                     tile_position=(0, 0))
                S = sp.tile([128, 128], BF16, tag=f"S{s}")
                nc.scalar.activation(S[:, 0:96], G[:, 0:96], SIG)
                nc.scalar.activation(S[:, 96:128], G[:, 96:128], SIG)
                U2 = sp.tile([128, ENC_F], BF16, tag=f"U2{s}")
                nc.vector.scalar_tensor_tensor(
                    U2[:], S[:, 64:96], 0.5, S[:, 32:64], SUB, MULT)
                C2 = sp.tile([128, ENC_F], BF16, tag=f"C2{s}")
                nc.vector.tensor_mul(C2[:], S[:, 0:32], C[s][:])
                nc.vector.tensor_add(C[s][:], U2[:], C2[:])
                T2 = sp.tile([128, ENC_F], BF16, tag=f"T2{s}")
                nc.scalar.activation(T2[:], C[s][:], TANH, scale=2.0)
                nc.vector.tensor_mul(H[s][:], T2[:], S[:, 96:128])

        # ---------------- enc->dec: xgd ----------------
        XG = [st.tile([128, 64], BF16, tag=f"XG{s}", name=f"XG{s}")
              for s in range(NSTREAM)]
        Hd = [st.tile([128, DEC_F], BF16, tag=f"Hd{s}", name=f"Hd{s}")
              for s in range(NSTREAM)]
        Cd = [st.tile([128, DEC_F], BF16, tag=f"Cd{s}", name=f"Cd{s}")
              for s in range(NSTREAM)]
        for s in range(NSTREAM):
            XGP = gp.tile([128, 128], F32, tag=f"G{s}", name=f"XGP{s}")
            for gi in range(4):
                for jh in range(2):
                    nc.tensor.matmul(
                        XGP[:, gi * DEC_F:(gi + 1) * DEC_F],
                        lT(O_WXGD, gi * 2 + jh),
                        H[s][:, jh * DEC_F:(jh + 1) * DEC_F],
                        start=(jh == 0), stop=(jh == 1),
                        tile_position=(0, 0))
            nc.vector.tensor_add(XG[s][:], XGP[:, 0:64], WF[:, 0:64])
            nc.vector.memset(Hd[s][:], 0.0)
            nc.vector.memset(Cd[s][:], 0.0)

        # ---------------- decoder ----------------
        Y = [None] * NSTREAM
        for t in range(T):
            j = t % 4
            tg = t // 4
            for s in range(NSTREAM):
                G = gp.tile([128, 64], F32, tag=f"Gd{s}", name=f"Gd{s}")
                nc.tensor.matmul(G[:], lT(O_ID, 0), XG[s][:],
                                 start=True, stop=False, tile_position=(0, 0))
                for gi in range(4):
                    nc.tensor.matmul(G[:, gi * DEC_F:(gi + 1) * DEC_F],
                                     lT(O_WHD, gi), Hd[s][:],
                                     start=False, stop=(gi == 3),
                                     tile_position=(0, 0))
                S = sp.tile([128, 64], BF16, tag=f"Sd{s}")
                nc.scalar.activation(S[:, 0:48], G[:, 0:48], SIG)
                nc.scalar.activation(S[:, 48:64], G[:, 48:64], SIG)
                U2 = sp.tile([128, DEC_F], BF16, tag=f"U2d{s}")
                nc.vector.scalar_tensor_tensor(
                    U2[:], S[:, 32:48], 0.5, S[:, 16:32], SUB, MULT)
                C2 = sp.tile([128, DEC_F], BF16, tag=f"C2d{s}")
                nc.vector.tensor_mul(C2[:], S[:, 0:16], Cd[s][:])
                nc.vector.tensor_add(Cd[s][:], U2[:], C2[:])
                T2 = sp.tile([128, DEC_F], BF16, tag=f"T2d{s}")
                nc.scalar.activation(T2[:], Cd[s][:], TANH, scale=2.0)
                nc.vector.tensor_mul(Hd[s][:], T2[:], S[:, 48:64])
                if j == 0:
                    Y[s] = yp.tile([128, 64], F32, tag=f"Y{s}", name=f"Y{s}")
                nc.tensor.matmul(Y[s][:, j * DEC_F:(j + 1) * DEC_F],
                                 lT(O_WY, 0), Hd[s][:],
                                 start=True, stop=True, tile_position=(0, 0))
                if j == 3:
                    nc.vector.tensor_scalar_add(
                        Ybuf[s][:, tg * 64:(tg + 1) * 64], Y[s][:],
                        WF[:, 64:65])
                    if (tg + 1) % (NG // 4) == 0:
                        h = (tg + 1) // (NG // 4) - 1
                        c0 = h * (NG // 4) * 64
                        c1 = (h + 1) * (NG // 4) * 64
                        nc.sync.dma_start(ydev[s, :, c0:c1],
                                          Ybuf[s][:, c0:c1])

    nc.compile()
    return nc


_cached = {}
TRACE = False
RUN_KWARGS = {}
LAST_RESULT = None


def _get_program(T=SEQ_LEN):
    if T not in _cached:
        _cached[T] = build_program(T)
    return _cached[T]


def kernel(x, enc_Wih, enc_Whh, enc_bih, enc_bhh,
           dec_Wih, dec_Whh, dec_bih, dec_bhh, out_W, out_b):
    from concourse.bass_utils import run_bass_kernel_spmd

    x = np.asarray(x, dtype=np.float32)
    T = x.shape[1]
    nc = _get_program(T)

    wb, wf = pack_weights(
        np.asarray(enc_Wih), np.asarray(enc_Whh),
        np.asarray(enc_bih), np.asarray(enc_bhh),
        np.asarray(dec_Wih), np.asarray(dec_Whh),
        np.asarray(dec_bih), np.asarray(dec_bhh),
        np.asarray(out_W), np.asarray(out_b))
    xdevs = prep_x(x, T)
    in_maps = [{"xdev": xdevs[c], "wblob": wb, "wf32": wf}
               for c in range(N_CORES)]
    res = run_bass_kernel_spmd(nc, in_maps, core_ids=list(range(N_CORES)),
                               trace=TRACE, **RUN_KWARGS)
    global LAST_RESULT
    LAST_RESULT = res
    return assemble_y([r["ydev"] for r in res.results], T)
